# revision 10
# baseline (speedup 1.0000x reference)
"""Behler-Parrinello NN (moe_routing) Trainium2 kernel.

Strategy:
  - Data-parallel over batch B=512 across 8 NeuronCores (64 rows each).
  - Atoms are routed: sorted by type into type-pure "chunks" of 8 atoms
    (x 64 batch rows = 512 tokens), padded with zero-atoms to chunk
    boundaries.  Each chunk runs through its own type's MLP weights, so
    compute is 1/T of the reference's all-types evaluation.
  - Per 128x512 SBUF tile we stack two chunks (top/bottom 64 partitions)
    and use tensor-engine tile_position packing so two 64x64 matmuls run
    concurrently in the 128x128 array.
  - Layer 3 (H2 -> 1) matmuls accumulate into a single persistent PSUM
    bank across the whole kernel, which implements the sum over atoms for
    free; the per-batch reduction finishes on the host (tiny).
  - silu(W x + b) is fused on ScalarE via activation(Silu, bias=...).
"""

import os
import sys

for _p in ("/opt/trn_rl_repo", "/root/.axon_site/_ro/trn_rl_repo"):
    if os.path.isdir(_p) and _p not in sys.path:
        sys.path.insert(0, _p)

import numpy as np

import concourse.bass as bass
import concourse.tile as tile
from concourse import bacc, mybir
from concourse.bass import ts
from concourse.bass_utils import run_bass_kernel_spmd

B, N, F, T, H1, H2 = 512, 2048, 64, 4, 64, 32
NCORES = 8
BC = B // NCORES          # 64 batch rows per core
CA = 8                    # atoms per chunk; CA * BC = 512 tokens per chunk
F32 = mybir.dt.float32
BF16 = mybir.dt.bfloat16

# test.py can read these after a traced run
LAST_EXEC_NS = None
LAST_RESULTS = None


def _ensure_ntff_hook():
    """This image's antenv lacks axon_hooks; synthesize it and install the
    ctypes NTFF profile hook from trn_agent_boot so trace=True works."""
    import importlib.util
    import types

    if importlib.util.find_spec("antenv.axon_hooks") is not None:
        return
    import antenv

    mod = types.ModuleType("antenv.axon_hooks")
    mod._hook = None
    mod.set_axon_ntff_profile_hook = lambda h: setattr(mod, "_hook", h)
    mod.get_axon_ntff_profile_hook = lambda: mod._hook
    sys.modules["antenv.axon_hooks"] = mod
    antenv.axon_hooks = mod
    try:
        from trn_agent_boot.trn_boot import _ntff_profile_via_ctypes

        mod._hook = _ntff_profile_via_ctypes("/opt/axon/libaxon_pjrt.so")
    except Exception as e:  # degrade to no-trace
        print(f"ntff hook install failed: {e}", file=sys.stderr)


def _chunk_schedule(an):
    """Sort atoms by type, pad each type to a chunk multiple, pad chunk count
    to a multiple of 4 (one quad = 4 chunks).  Returns (slots, ctype,
    counts, pad_counts): slots is [nchunks*CA] atom indices with -1 = pad."""
    order = np.argsort(an, kind="stable")
    counts = np.bincount(an, minlength=T).astype(np.int64)
    slots = []
    ctype = []
    pad_counts = np.zeros(T, dtype=np.int64)
    pos = 0
    for t in range(T):
        idx = order[pos : pos + counts[t]]
        pos += counts[t]
        nch = (counts[t] + CA - 1) // CA
        padded = np.full(nch * CA, -1, dtype=np.int64)
        padded[: counts[t]] = idx
        pad_counts[t] += nch * CA - counts[t]
        slots.append(padded)
        ctype.extend([t] * int(nch))
    while len(ctype) % 8 != 0:
        slots.append(np.full(CA, -1, dtype=np.int64))
        pad_counts[T - 1] += CA
        ctype.append(T - 1)
    return np.concatenate(slots), np.array(ctype, dtype=np.int64), counts, pad_counts


def gen_bass(nchunks, ctype, pair_combo, n_used):
    """Build the per-core Bass kernel.  ctype (len nchunks, multiple of 8) and
    the per-pair combo table index are baked in at compile time."""
    npairs = nchunks // 2
    nquads = nchunks // 4
    ngroups = nquads // 2
    Silu = mybir.ActivationFunctionType.Silu

    nc = bacc.Bacc(None, target_bir_lowering=False)
    xt4 = nc.dram_tensor("xt4", [nquads, 128, 1024], BF16, kind="ExternalInput")
    WCOLS = n_used * (128 + 64 + 64)
    wad = nc.dram_tensor("wall", [128, WCOLS], BF16, kind="ExternalInput")
    bad = nc.dram_tensor("ball", [128, npairs + nquads], F32, kind="ExternalInput")
    outd = nc.dram_tensor("out", [4, 1024], F32, kind="ExternalOutput")

    with tile.TileContext(nc) as tc:
        with (
            tc.tile_pool(name="consts", bufs=1) as cpool,
            tc.tile_pool(name="xp", bufs=6) as xpool,
            tc.tile_pool(name="h1p", bufs=3) as h1pool,
            tc.tile_pool(name="h2p", bufs=2) as h2pool,
            tc.tile_pool(name="outp", bufs=1) as opool,
            tc.tile_pool(name="ps1", bufs=2, space="PSUM") as ps1pool,
            tc.tile_pool(name="ps23", bufs=2, space="PSUM") as ps23pool,
        ):
            # prefetch the first quads' x tiles so compute can start while
            # the (larger) weight load still streams
            xpre = {}
            for q in range(min(4, nquads)):
                xq = xpool.tile([128, 1024], BF16, tag="x", name=f"xpre{q}")
                nc.sync.dma_start(xq[:], xt4[q])
                xpre[q] = xq

            # one combined weight tile, loaded via partition-range-split DMAs
            # (large per-descriptor payloads, parallel queues -> short head)
            wat = cpool.tile([128, WCOLS], BF16)
            for r in range(8):
                nc.sync.dma_start(wat[16 * r : 16 * r + 16, :],
                                  wad[16 * r : 16 * r + 16, :])
            bat = cpool.tile([128, npairs + nquads], F32)
            for r in range(2):
                nc.sync.dma_start(bat[64 * r : 64 * r + 64, :],
                                  bad[64 * r : 64 * r + 64, :])
            w0t = wat[:, 0 : n_used * 128]
            w1t = wat[:, n_used * 128 : n_used * 192]
            w2t = wat[:, n_used * 192 : n_used * 256]
            b0t = bat[:, 0:npairs]
            b1t = bat[:, npairs : npairs + nquads]

            acc = opool.tile([128, 1024], F32)
            nc.vector.memset(acc[:], 0.0)

            for g in range(ngroups):
                h1s = []
                ps23 = ps23pool.tile([128, 1024], F32, tag="ps23")
                h2 = h2pool.tile([128, 1024], BF16, tag="h2")
                for j in range(2):
                    q = 2 * g + j
                    p0, p1 = 2 * q, 2 * q + 1
                    cAB, cCD = pair_combo[p0], pair_combo[p1]
                    tA, tB = ctype[2 * p0], ctype[2 * p0 + 1]
                    tC, tD = ctype[2 * p1], ctype[2 * p1 + 1]
                    if q in xpre:
                        x01 = xpre.pop(q)
                    else:
                        x01 = xpool.tile([128, 1024], BF16, tag="x")
                        nc.sync.dma_start(x01[:], xt4[q])

                    # L1: one K=128, M=128 matmul per chunk-pair
                    ps1 = ps1pool.tile([128, 1024], F32, tag="ps1")
                    nc.tensor.matmul(ps1[:, 0:512], w0t[:, ts(cAB, 128)],
                                     x01[:, 0:512], start=True, stop=True,
                                     tile_position=(0, 0))
                    nc.tensor.matmul(ps1[:, 512:1024], w0t[:, ts(cCD, 128)],
                                     x01[:, 512:1024], start=True, stop=True,
                                     tile_position=(0, 0))

                    h1 = h1pool.tile([128, 1024], BF16, tag="h1")
                    if (tA, tB) == (tC, tD):
                        nc.scalar.activation(h1[:], ps1[:], Silu,
                                             bias=b0t[:, p0 : p0 + 1])
                    else:
                        nc.scalar.activation(h1[:, 0:512], ps1[:, 0:512], Silu,
                                             bias=b0t[:, p0 : p0 + 1])
                        nc.scalar.activation(h1[:, 512:1024], ps1[:, 512:1024],
                                             Silu, bias=b0t[:, p1 : p1 + 1])
                    h1s.append((h1, cAB, cCD))

                    # L2: one K=128, M=64 matmul per pair -> ps23 half
                    nc.tensor.matmul(ps23[0:64, ts(j, 512)], w1t[:, ts(cAB, 64)],
                                     h1[:, 0:512], start=True, stop=True,
                                     tile_position=(0, 0))
                    nc.tensor.matmul(ps23[64:128, ts(j, 512)], w1t[:, ts(cCD, 64)],
                                     h1[:, 512:1024], start=True, stop=True,
                                     tile_position=(0, 64))

                # L2 act over both quads at once when their type tuple matches
                q0, q1 = 2 * g, 2 * g + 1
                if list(ctype[8 * g : 8 * g + 4]) == list(ctype[8 * g + 4 : 8 * g + 8]):
                    nc.scalar.activation(h2[:], ps23[:], Silu,
                                         bias=b1t[:, q0 : q0 + 1])
                else:
                    nc.scalar.activation(h2[:, 0:512], ps23[:, 0:512], Silu,
                                         bias=b1t[:, q0 : q0 + 1])
                    nc.scalar.activation(h2[:, 512:1024], ps23[:, 512:1024], Silu,
                                         bias=b1t[:, q1 : q1 + 1])

                # L3 reuses ps23 after the act has drained it (WAR via Tile)
                for j in range(2):
                    _, cAB, cCD = h1s[j]
                    nc.tensor.matmul(ps23[0:64, ts(j, 512)], w2t[0:64, ts(cAB, 64)],
                                     h2[0:64, ts(j, 512)], start=True, stop=True,
                                     tile_position=(0, 0))
                    nc.tensor.matmul(ps23[64:128, ts(j, 512)],
                                     w2t[64:128, ts(cCD, 64)],
                                     h2[64:128, ts(j, 512)], start=True, stop=True,
                                     tile_position=(64, 64))
                nc.vector.tensor_add(out=acc[:], in0=acc[:], in1=ps23[:])

            for i, p in enumerate((0, 32, 64, 96)):
                nc.sync.dma_start(outd[i : i + 1, :], acc[p : p + 1, :])
    nc.finalize()
    return nc


def _prep_core_x(x_c, slots, mask, npairs):
    """[BC, N, F] full-precision batch shard -> [npairs, 128, CA*BC] tiles.
    Tile p partition h*F+f, column a*BC+b = x_c[b, slots[(2p+h)*CA+a], f]."""
    xg = x_c[:, np.where(mask, slots, 0), :]          # [BC, NS, F]
    xg[:, ~mask, :] = 0.0
    nchunks = slots.shape[0] // CA
    xg = np.ascontiguousarray(xg.transpose(1, 2, 0))  # [NS, F, BC]
    xg = xg.reshape(nchunks, CA, F, BC).transpose(0, 2, 1, 3)  # [ch, F, CA, BC]
    return np.ascontiguousarray(xg).reshape(npairs, 2 * F, CA * BC)


def kernel(x, atomic_numbers, w0, b0, w1, b1, w2, b2, trace=False):
    global LAST_EXEC_NS, LAST_RESULTS
    x = np.asarray(x, dtype=np.float32)
    an = np.asarray(atomic_numbers).astype(np.int64)
    w0 = np.asarray(w0, dtype=np.float32)
    b0 = np.asarray(b0, dtype=np.float32)
    w1 = np.asarray(w1, dtype=np.float32)
    b1 = np.asarray(b1, dtype=np.float32)
    w2 = np.asarray(w2, dtype=np.float32)
    b2 = np.asarray(b2, dtype=np.float32)

    slots, ctype, counts, pad_counts = _chunk_schedule(an)
    nchunks = len(ctype)
    npairs, nquads = nchunks // 2, nchunks // 4
    mask = slots >= 0

    # --- device-side weight/bias layouts (shared across cores) ---
    # only ship the type-combos that actually occur
    import ml_dtypes

    bf16 = ml_dtypes.bfloat16
    pairs_t = [(int(ctype[2 * p]), int(ctype[2 * p + 1])) for p in range(npairs)]
    used = sorted(set(pairs_t))
    n_used = len(used)
    cmap = {c: i for i, c in enumerate(used)}
    pair_combo = [cmap[c] for c in pairs_t]
    w0s = np.zeros((128, n_used * 128), dtype=np.float32)
    w1s = np.zeros((128, n_used * 64), dtype=np.float32)
    w2s = np.zeros((128, n_used * 64), dtype=np.float32)
    for (tt, tb), c in cmap.items():
        w0s[0:64, c * 128 : c * 128 + 64] = w0[tt].T
        w0s[64:128, c * 128 + 64 : c * 128 + 128] = w0[tb].T
        w1s[0:64, c * 64 : c * 64 + 32] = w1[tt].T
        w1s[64:128, c * 64 + 32 : c * 64 + 64] = w1[tb].T
        for half in (0, 64):
            w2s[half : half + 32, c * 64] = w2[tt, 0, :]
            w2s[half + 32 : half + 64, c * 64 + 32] = w2[tb, 0, :]
    b0p = np.zeros((128, npairs), dtype=np.float32)
    for p in range(npairs):
        b0p[0:64, p] = b0[ctype[2 * p]]
        b0p[64:128, p] = b0[ctype[2 * p + 1]]
    b1q = np.zeros((128, nquads), dtype=np.float32)
    for q in range(nquads):
        tA, tB, tC, tD = ctype[4 * q : 4 * q + 4]
        b1q[0:32, q] = b1[tA]
        b1q[32:64, q] = b1[tB]
        b1q[64:96, q] = b1[tC]
        b1q[96:128, q] = b1[tD]
    wall = np.concatenate([w0s, w1s, w2s], axis=1).astype(bf16)
    ball = np.concatenate([b0p, b1q], axis=1).astype(np.float32)
    shared = {"wall": wall, "ball": ball}
    in_maps = []
    for c in range(NCORES):
        xt3 = _prep_core_x(x[c * BC : (c + 1) * BC], slots, mask, npairs).astype(bf16)
        xt4 = np.ascontiguousarray(
            xt3.reshape(nquads, 2, 128, CA * BC).transpose(0, 2, 1, 3)
        ).reshape(nquads, 128, 2 * CA * BC)
        in_maps.append({"xt4": xt4, **shared})

    if trace:
        _ensure_ntff_hook()
    nc = gen_bass(nchunks, ctype, pair_combo, n_used)
    res = run_bass_kernel_spmd(nc, in_maps, core_ids=list(range(NCORES)),
                               trace=trace)
    LAST_EXEC_NS = res.exec_time_ns
    LAST_RESULTS = res

    # --- host-side unshard + tiny corrections ---
    # device out = sum over streamed tokens of w2 . h2(token); pads
    # contribute e0[t] = w2[t] . silu(w1[t] silu(b0[t]) + b1[t]); real atoms
    # still owe their +b2[t].
    def _silu(v):
        return v / (1.0 + np.exp(-v))

    e0 = np.zeros(T, dtype=np.float64)
    for t in range(T):
        h1v = _silu(b0[t].astype(np.float64))
        h2v = _silu(w1[t].astype(np.float64) @ h1v + b1[t])
        e0[t] = w2[t, 0] @ h2v
    bias_term = float((counts * b2[:, 0].astype(np.float64)).sum())
    pad_term = float((pad_counts * e0).sum())

    out = np.empty(B, dtype=np.float32)
    for c in range(NCORES):
        dev = res.results[c]["out"]                   # [4, 1024]
        s = dev.sum(axis=0).reshape(2, CA, BC).sum(axis=(0, 1))
        out[c * BC : (c + 1) * BC] = s + bias_term - pad_term
    return out
